# revision 49
# baseline (speedup 1.0000x reference)
"""Trainium2 Bass kernel for MeshInterpolate (interpolate_face_attributes).

Problem (hardcoded shapes):
  pix_to_face [4, 512, 512, 1] int64 (-1 = background), values in [-1, 10000)
  bary_coords [4, 512, 512, 1, 3] f32
  face_memory [10000, 3, 128] f32
  output      [4, 128, 512, 512] f32 (NCHW)

Sharding: data-parallel over (N, H/2): 8 cores, core c handles image c//2,
rows 256*(c%2) .. +256  -> 131072 pixels per core. face_memory replicated.

Design (all bf16 on device; host converts in/out, rel-err gate is 2e-2;
measured ~465-530us (device-load dependent) vs 1257us f32 baseline):
  - face_memory cast to bf16 -> 768 B gather rows (half the HBM traffic of
    f32).  Background pixels index a zero row appended at fm[10000].
  - gathers are 1024 idxs each (SWDGE descriptor ring holds exactly 1024
    16B descriptors) rotated across 4 SWDGE queues: the dma_gather ucode
    runs on the Q7 core pair selected by queue_num, so rotation pipelines
    descriptor generation across 4 pairs (10.4us -> ~3.2us per call).
  - per compute chunk of 2048 pixels (2 gathers): ONE tensor_tensor does
    all 3*2048*128 products: prod[p,g,v,c] = attrs[p,g,(v c)] * bary
    (instead of 48 tensor_scalar [128,128] ops; DVE was 95% busy).  bary is
    host-replicated 8x so the operand has a contiguous 8-elem inner run
    plus a stride-0 middle dim: a pure stride-0 broadcast blocks the DVE
    2x 16-bit mode (measured 6.6us -> 3.4us per chunk op).
  - PE does transpose AND the vertex sum in one pass: per 128-pixel block,
    3 accumulating matmuls psum[c,p] += prod_v^T @ I with f32 psum
    accumulation (bf16 operands = 1 cycle/row; no DVE adds needed).
  - psum/bounce/out run at half-chunk (1024 px) granularity: 2-bank psum
    tiles x 4 bufs keep the PE->ACT->out chain fine-grained so DMA work
    stays smooth and the per-queue SWDGE rings (1 call deep) drain without
    bursty stalls.
  - ACT copies psum f32 -> bf16 bounce; output DMA'd as bf16 (33.5 MB/core
    instead of 67), host upconverts to f32.
Remaining wall is the DMA-engine byte floor: 100.7 MB gather + 33.5 MB out
+ ~3.5 MB in = 138 MB/core at ~360 GB/s = ~390us, measured ~90% DMA active.
"""

import os

import numpy as np

# Safety: recover wedged NeuronCores from a previous crashed process. Must be
# set before the first jax/NRT backend init in this process.
os.environ.setdefault("NEURON_RT_RESET_CORES", "1")

P = 128
ELEM = 384            # one face row: 3*128 bf16 elements (768 B)
ESTEP = 512           # fm row stride in bf16 elems: 1 KB-aligned rows so no
                      # gather read straddles a 1 KB boundary (768 B payload)
GATHER = 1024         # pixels per dma_gather call (descriptor-ring carveout)
GPG = GATHER // P     # 8 g-blocks per gather
CHUNK = 2048          # pixels per compute chunk (2 gathers)
GPC = CHUNK // P      # 16 g-blocks per chunk
NCHPS = 4             # chunks per supertile (idx/bary load granularity)
SUPER = CHUNK * NCHPS # 8192 pixels
F = 10000
N_CORES = 8
NPIX_CORE = 131072
NSUPER = NPIX_CORE // SUPER  # 16

_CACHE = {}


def _build_nc(nsuper=NSUPER):
    import concourse.bacc as bacc
    import concourse.mybir as mybir
    from concourse import tile
    from concourse.library_config import mlp

    npix = nsuper * SUPER
    # 4 SWDGE queues: dma_gather descriptor generation runs on a Q7 core pair
    # selected by queue_num, so rotating queues pipelines generation across 4
    # core pairs (measured 10.4us -> ~4.8us per 1024-idx gather).
    nc = bacc.Bacc("TRN2", target_bir_lowering=False, debug=False,
                   num_swdge_queues=4)
    fm = nc.dram_tensor("fm", [F + 1, ESTEP], mybir.dt.bfloat16, kind="ExternalInput")
    nch = nsuper * NCHPS
    idxw = nc.dram_tensor("idxw", [nch, P, CHUNK // 16],
                          mybir.dt.int16, kind="ExternalInput")
    # bary replicated 8x on host: gives the product tensor_tensor a
    # contiguous 8-elem inner run (pure stride-0 broadcast blocks the DVE
    # 2x 16-bit mode); on-chip expansion was tried and costs ~4us/chunk of
    # DVE time vs ~15us total of DMA, so shipping it is cheaper
    baryt = nc.dram_tensor("baryt", [nch, P, GPC * 3, 8],
                           mybir.dt.bfloat16, kind="ExternalInput")
    ident = nc.dram_tensor("ident", [P, P], mybir.dt.bfloat16, kind="ExternalInput")
    out = nc.dram_tensor("out", [P, npix], mybir.dt.bfloat16, kind="ExternalOutput")

    cw = GATHER // 16  # idx columns per gather call
    with tile.TileContext(nc) as tc:
        nc.gpsimd.load_library(mlp)
        with (
            tc.tile_pool(name="const", bufs=1) as constp,
            tc.tile_pool(name="io", bufs=6) as iop,
            tc.tile_pool(name="attrs", bufs=6) as attrp,
            tc.tile_pool(name="prod", bufs=3) as prodp,
            tc.tile_pool(name="bounce", bufs=4) as bouncep,
            tc.tile_pool(name="ps", bufs=4, space="PSUM") as psump,
        ):
            id_sb = constp.tile([P, P], mybir.dt.bfloat16, tag="ident")
            nc.sync.dma_start(id_sb[:], ident[:])
            for ch in range(nch):
                # per-chunk idx/bary loads (32 KB + 12 KB): the first gather
                # only waits for its own chunk's indices, and input DMA
                # arrives smoothly instead of in per-supertile bursts
                idx_sb = iop.tile([P, CHUNK // 16], mybir.dt.int16, tag="idx")
                bary8 = iop.tile([P, GPC * 3, 8], mybir.dt.bfloat16, tag="bary")
                nc.sync.dma_start(idx_sb[:], idxw[ch])
                nc.sync.dma_start(bary8[:], baryt[ch])
                attrs = attrp.tile([P, GPC, ELEM], mybir.dt.bfloat16, tag="attrs")
                with tc.high_priority(offset=400):
                    for h in range(2):
                        nc.gpsimd.dma_gather(
                            attrs[:, h * GPG:(h + 1) * GPG, :],
                            fm[:, :ELEM],
                            idx_sb[:, h * cw:(h + 1) * cw],
                            GATHER, GATHER, ELEM, elem_step=ESTEP,
                            queue_num=(ch * 2 + h) % 4)
                prod = prodp.tile([P, GPC, 3, P], mybir.dt.bfloat16, tag="prod")
                a4 = attrs[:].rearrange("p g (v a b) -> p (g v) a b", v=3, b=8)
                b4 = (bary8[:, :, :]
                      .unsqueeze(2).broadcast_to((P, GPC * 3, P // 8, 8)))
                p4 = prod[:].rearrange("p g v (a b) -> p (g v) a b", b=8)
                nc.vector.tensor_tensor(p4, a4, b4, mybir.AluOpType.mult)
                # PE does transpose AND the v-sum: psum[c, p] += prod_v^T @ I
                # (normal matmul, f32 psum accumulation over the 3 vertices)
                # Half-chunk psum tiles (2 banks x 4 bufs) keep the
                # PE->ACT->out pipeline fine-grained so DMA-engine work
                # stays smooth and SWDGE rings drain without bursts.
                for h in range(2):
                    ps = psump.tile([P, GATHER], mybir.dt.float32, tag="ps")
                    for gg in range(GPG):
                        for v in range(3):
                            nc.tensor.matmul(
                                ps[:, gg * P:(gg + 1) * P],
                                prod[:, h * GPG + gg, v, :],
                                id_sb[:], start=(v == 0), stop=(v == 2))
                    bounce = bouncep.tile([P, GATHER], mybir.dt.bfloat16,
                                          tag="bounce")
                    nc.scalar.copy(bounce[:], ps[:])
                    col = ch * CHUNK + h * GATHER
                    # out-DMA issued from ACT's own HWDGE: keeps the Sync
                    # queue free for idx/bary loads (which gate gathers) and
                    # avoids a cross-engine handoff after the psum copy
                    nc.scalar.dma_start(out[:, col:col + GATHER], bounce[:])
    nc.compile()
    return nc


def _get_nc():
    if "nc" not in _CACHE:
        _CACHE["nc"] = _build_nc()
    return _CACHE["nc"]


def _prep_in_maps(pix_to_face, bary_coords, face_memory):
    import ml_dtypes

    bf16 = ml_dtypes.bfloat16
    N, H, W, K = pix_to_face.shape          # 4, 512, 512, 1
    assert (N, H, W, K) == (4, 512, 512, 1)
    fm = np.asarray(face_memory, dtype=np.float32).reshape(F, ELEM)
    fm_pad = np.zeros((F + 1, ESTEP), np.float32)
    fm_pad[:F, :ELEM] = fm
    fm_pad = fm_pad.astype(bf16)
    ident = np.eye(P, dtype=np.float32).astype(bf16)

    idx_all = np.asarray(pix_to_face).reshape(N, H, W)
    bary_all = np.asarray(bary_coords, dtype=np.float32).reshape(N, H, W, 3)

    ngath = NPIX_CORE // GATHER  # 128 gather groups per core
    gps = SUPER // GATHER        # 8 gather groups per supertile
    in_maps = []
    for c in range(N_CORES):
        n, hh = c // 2, (c % 2) * 256
        idx = idx_all[n, hh:hh + 256].reshape(-1)                 # [131072]
        bary = bary_all[n, hh:hh + 256].reshape(-1, 3)            # [131072, 3]
        idx16 = np.where(idx < 0, F, idx).astype(np.int16)
        # per gather group: wrap 16-way ([16, GATHER/16]), replicate to 128
        # partitions; groups laid side by side per supertile along free dim
        nch = NPIX_CORE // CHUNK  # 64 chunks per core, 2 gather groups each
        idxw = np.ascontiguousarray(
            idx16.reshape(ngath, GATHER // 16, 16).transpose(0, 2, 1))
        idxw = np.tile(idxw, (1, 8, 1))                  # [128, 128, GATHER/16]
        idxw = np.ascontiguousarray(
            idxw.reshape(nch, 2, P, GATHER // 16)
            .transpose(0, 2, 1, 3)
            .reshape(nch, P, CHUNK // 16))
        baryt = np.ascontiguousarray(
            bary.reshape(ngath, GPG, P, 3).transpose(0, 2, 1, 3)  # [128,128,8,3]
            .reshape(nch, 2, P, GPG, 3)
            .transpose(0, 2, 1, 3, 4)
            .reshape(nch, P, GPC * 3, 1)).astype(bf16)
        baryt = np.ascontiguousarray(np.repeat(baryt, 8, axis=3))
        in_maps.append({"fm": fm_pad, "idxw": idxw, "baryt": baryt, "ident": ident})
    return in_maps


def _assemble(results):
    out_full = np.empty((4, 128, 512, 512), dtype=np.float32)
    for c in range(N_CORES):
        n, hh = c // 2, (c % 2) * 256
        out_full[n, :, hh:hh + 256, :] = (
            results[c]["out"].astype(np.float32).reshape(128, 256, 512))
    return out_full


def run(in_maps, trace=False, trace_kwargs=None):
    from concourse.bass_utils import run_bass_kernel_spmd

    nc = _get_nc()
    kw = {}
    if trace:
        kw = dict(trace=True, trace_kwargs=trace_kwargs or {})
    return run_bass_kernel_spmd(nc, in_maps, list(range(N_CORES)), **kw)


def kernel(pix_to_face, bary_coords, face_memory):
    in_maps = _prep_in_maps(pix_to_face, bary_coords, face_memory)
    res = run(in_maps)
    return _assemble(res.results)


# revision 50
# speedup vs baseline: 1.1287x; 1.1287x over previous
"""Trainium2 Bass kernel for MeshInterpolate (interpolate_face_attributes).

Problem (hardcoded shapes):
  pix_to_face [4, 512, 512, 1] int64 (-1 = background), values in [-1, 10000)
  bary_coords [4, 512, 512, 1, 3] f32
  face_memory [10000, 3, 128] f32
  output      [4, 128, 512, 512] f32 (NCHW)

Sharding: data-parallel over (N, H/2): 8 cores, core c handles image c//2,
rows 256*(c%2) .. +256  -> 131072 pixels per core. face_memory replicated.

Design (all bf16 on device; host converts in/out, rel-err gate is 2e-2;
measured ~465-530us (device-load dependent) vs 1257us f32 baseline):
  - face_memory cast to bf16 -> 768 B gather rows (half the HBM traffic of
    f32).  Background pixels index a zero row appended at fm[10000].
  - gathers are 1024 idxs each (SWDGE descriptor ring holds exactly 1024
    16B descriptors) rotated across 4 SWDGE queues: the dma_gather ucode
    runs on the Q7 core pair selected by queue_num, so rotation pipelines
    descriptor generation across 4 pairs (10.4us -> ~3.2us per call).
  - per compute chunk of 2048 pixels (2 gathers): ONE tensor_tensor does
    all 3*2048*128 products: prod[p,g,v,c] = attrs[p,g,(v c)] * bary
    (instead of 48 tensor_scalar [128,128] ops; DVE was 95% busy).  bary is
    host-replicated 8x so the operand has a contiguous 8-elem inner run
    plus a stride-0 middle dim: a pure stride-0 broadcast blocks the DVE
    2x 16-bit mode (measured 6.6us -> 3.4us per chunk op).
  - PE does transpose AND the vertex sum in one pass: per 128-pixel block,
    3 accumulating matmuls psum[c,p] += prod_v^T @ I with f32 psum
    accumulation (bf16 operands = 1 cycle/row; no DVE adds needed).
  - psum/bounce/out run at half-chunk (1024 px) granularity: 2-bank psum
    tiles x 4 bufs keep the PE->ACT->out chain fine-grained so DMA work
    stays smooth and the per-queue SWDGE rings (1 call deep) drain without
    bursty stalls.
  - ACT copies psum f32 -> bf16 bounce; output DMA'd as bf16 (33.5 MB/core
    instead of 67), host upconverts to f32.
Remaining wall is the DMA-engine byte floor: 100.7 MB gather + 33.5 MB out
+ ~3.5 MB in = 138 MB/core at ~360 GB/s = ~390us, measured ~90% DMA active.
"""

import os

import numpy as np

# Safety: recover wedged NeuronCores from a previous crashed process. Must be
# set before the first jax/NRT backend init in this process.
os.environ.setdefault("NEURON_RT_RESET_CORES", "1")

P = 128
ELEM = 384            # one face row: 3*128 bf16 elements (768 B)
ESTEP = 512           # fm row stride in bf16 elems: 1 KB-aligned rows so no
                      # gather read straddles a 1 KB boundary (768 B payload)
GATHER = 1024         # pixels per dma_gather call (descriptor-ring carveout)
GPG = GATHER // P     # 8 g-blocks per gather
CHUNK = 2048          # pixels per compute chunk (2 gathers)
GPC = CHUNK // P      # 16 g-blocks per chunk
NCHPS = 4             # chunks per supertile (idx/bary load granularity)
SUPER = CHUNK * NCHPS # 8192 pixels
F = 10000
N_CORES = 8
NPIX_CORE = 131072
NSUPER = NPIX_CORE // SUPER  # 16

_CACHE = {}


def _build_nc(nsuper=NSUPER):
    import concourse.bacc as bacc
    import concourse.mybir as mybir
    from concourse import tile
    from concourse.library_config import mlp

    npix = nsuper * SUPER
    # 4 SWDGE queues: dma_gather descriptor generation runs on a Q7 core pair
    # selected by queue_num, so rotating queues pipelines generation across 4
    # core pairs (measured 10.4us -> ~4.8us per 1024-idx gather).
    nc = bacc.Bacc("TRN2", target_bir_lowering=False, debug=False,
                   num_swdge_queues=4)
    fm = nc.dram_tensor("fm", [F + 1, ESTEP], mybir.dt.bfloat16, kind="ExternalInput")
    nch = nsuper * NCHPS
    idxw = nc.dram_tensor("idxw", [nch, P, CHUNK // 16],
                          mybir.dt.int16, kind="ExternalInput")
    # bary replicated 8x on host: gives the product tensor_tensor a
    # contiguous 8-elem inner run (pure stride-0 broadcast blocks the DVE
    # 2x 16-bit mode); on-chip expansion was tried and costs ~4us/chunk of
    # DVE time vs ~15us total of DMA, so shipping it is cheaper
    baryt = nc.dram_tensor("baryt", [nch, P, GPC * 3, 8],
                           mybir.dt.bfloat16, kind="ExternalInput")
    ident = nc.dram_tensor("ident", [P, P], mybir.dt.bfloat16, kind="ExternalInput")
    out = nc.dram_tensor("out", [P, npix], mybir.dt.bfloat16, kind="ExternalOutput")

    cw = GATHER // 16  # idx columns per gather call
    with tile.TileContext(nc) as tc:
        nc.gpsimd.load_library(mlp)
        with (
            tc.tile_pool(name="const", bufs=1) as constp,
            tc.tile_pool(name="io", bufs=6) as iop,
            tc.tile_pool(name="attrs", bufs=6) as attrp,
            tc.tile_pool(name="prod", bufs=3) as prodp,
            tc.tile_pool(name="bounce", bufs=4) as bouncep,
            tc.tile_pool(name="ps", bufs=4, space="PSUM") as psump,
        ):
            id_sb = constp.tile([P, P], mybir.dt.bfloat16, tag="ident")
            nc.sync.dma_start(id_sb[:], ident[:])
            for ch in range(nch):
                # per-chunk idx/bary loads (32 KB + 12 KB): the first gather
                # only waits for its own chunk's indices, and input DMA
                # arrives smoothly instead of in per-supertile bursts
                idx_sb = iop.tile([P, CHUNK // 16], mybir.dt.int16, tag="idx")
                bary8 = iop.tile([P, GPC * 3, 8], mybir.dt.bfloat16, tag="bary")
                nc.sync.dma_start(idx_sb[:], idxw[ch])
                nc.sync.dma_start(bary8[:], baryt[ch])
                attrs = attrp.tile([P, GPC, ELEM], mybir.dt.bfloat16, tag="attrs")
                with tc.high_priority(offset=400):
                    for h in range(2):
                        nc.gpsimd.dma_gather(
                            attrs[:, h * GPG:(h + 1) * GPG, :],
                            fm[:, :ELEM],
                            idx_sb[:, h * cw:(h + 1) * cw],
                            GATHER, GATHER, ELEM, elem_step=ESTEP,
                            queue_num=(ch * 2 + h) % 4)
                prod = prodp.tile([P, GPC, 3, P], mybir.dt.bfloat16, tag="prod")
                a4 = attrs[:].rearrange("p g (v a b) -> p (g v) a b", v=3, b=8)
                b4 = (bary8[:, :, :]
                      .unsqueeze(2).broadcast_to((P, GPC * 3, P // 8, 8)))
                p4 = prod[:].rearrange("p g v (a b) -> p (g v) a b", b=8)
                nc.vector.tensor_tensor(p4, a4, b4, mybir.AluOpType.mult)
                # PE does transpose AND the v-sum: psum[c, p] += prod_v^T @ I
                # (normal matmul, f32 psum accumulation over the 3 vertices)
                # Half-chunk psum tiles (2 banks x 4 bufs) keep the
                # PE->ACT->out pipeline fine-grained so DMA-engine work
                # stays smooth and SWDGE rings drain without bursts.
                for h in range(2):
                    ps = psump.tile([P, GATHER], mybir.dt.float32, tag="ps")
                    for gg in range(GPG):
                        for v in range(3):
                            nc.tensor.matmul(
                                ps[:, gg * P:(gg + 1) * P],
                                prod[:, h * GPG + gg, v, :],
                                id_sb[:], start=(v == 0), stop=(v == 2))
                    bounce = bouncep.tile([P, GATHER], mybir.dt.bfloat16,
                                          tag="bounce")
                    nc.scalar.copy(bounce[:], ps[:])
                    col = ch * CHUNK + h * GATHER
                    nc.sync.dma_start(out[:, col:col + GATHER], bounce[:])
    nc.compile()
    return nc


def _get_nc():
    if "nc" not in _CACHE:
        _CACHE["nc"] = _build_nc()
    return _CACHE["nc"]


def _prep_in_maps(pix_to_face, bary_coords, face_memory):
    import ml_dtypes

    bf16 = ml_dtypes.bfloat16
    N, H, W, K = pix_to_face.shape          # 4, 512, 512, 1
    assert (N, H, W, K) == (4, 512, 512, 1)
    fm = np.asarray(face_memory, dtype=np.float32).reshape(F, ELEM)
    fm_pad = np.zeros((F + 1, ESTEP), np.float32)
    fm_pad[:F, :ELEM] = fm
    fm_pad = fm_pad.astype(bf16)
    ident = np.eye(P, dtype=np.float32).astype(bf16)

    idx_all = np.asarray(pix_to_face).reshape(N, H, W)
    bary_all = np.asarray(bary_coords, dtype=np.float32).reshape(N, H, W, 3)

    ngath = NPIX_CORE // GATHER  # 128 gather groups per core
    gps = SUPER // GATHER        # 8 gather groups per supertile
    in_maps = []
    for c in range(N_CORES):
        n, hh = c // 2, (c % 2) * 256
        idx = idx_all[n, hh:hh + 256].reshape(-1)                 # [131072]
        bary = bary_all[n, hh:hh + 256].reshape(-1, 3)            # [131072, 3]
        idx16 = np.where(idx < 0, F, idx).astype(np.int16)
        # per gather group: wrap 16-way ([16, GATHER/16]), replicate to 128
        # partitions; groups laid side by side per supertile along free dim
        nch = NPIX_CORE // CHUNK  # 64 chunks per core, 2 gather groups each
        idxw = np.ascontiguousarray(
            idx16.reshape(ngath, GATHER // 16, 16).transpose(0, 2, 1))
        idxw = np.tile(idxw, (1, 8, 1))                  # [128, 128, GATHER/16]
        idxw = np.ascontiguousarray(
            idxw.reshape(nch, 2, P, GATHER // 16)
            .transpose(0, 2, 1, 3)
            .reshape(nch, P, CHUNK // 16))
        baryt = np.ascontiguousarray(
            bary.reshape(ngath, GPG, P, 3).transpose(0, 2, 1, 3)  # [128,128,8,3]
            .reshape(nch, 2, P, GPG, 3)
            .transpose(0, 2, 1, 3, 4)
            .reshape(nch, P, GPC * 3, 1)).astype(bf16)
        baryt = np.ascontiguousarray(np.repeat(baryt, 8, axis=3))
        in_maps.append({"fm": fm_pad, "idxw": idxw, "baryt": baryt, "ident": ident})
    return in_maps


def _assemble(results):
    out_full = np.empty((4, 128, 512, 512), dtype=np.float32)
    for c in range(N_CORES):
        n, hh = c // 2, (c % 2) * 256
        out_full[n, :, hh:hh + 256, :] = (
            results[c]["out"].astype(np.float32).reshape(128, 256, 512))
    return out_full


def run(in_maps, trace=False, trace_kwargs=None):
    from concourse.bass_utils import run_bass_kernel_spmd

    nc = _get_nc()
    kw = {}
    if trace:
        kw = dict(trace=True, trace_kwargs=trace_kwargs or {})
    return run_bass_kernel_spmd(nc, in_maps, list(range(N_CORES)), **kw)


def kernel(pix_to_face, bary_coords, face_memory):
    in_maps = _prep_in_maps(pix_to_face, bary_coords, face_memory)
    res = run(in_maps)
    return _assemble(res.results)
